# revision 8
# baseline (speedup 1.0000x reference)
import sys

for _p in ("/opt/trn_rl_repo", "/root/.axon_site/_ro/trn_rl_repo"):
    if _p not in sys.path:
        sys.path.append(_p)

import numpy as np

# Problem: B=8 batches of cross-attention-like softmax matmul, one batch per core.
#   S[e,t] = sum_d enc[e,d] * dec[t,d]
#   A = softmax(S, axis=t)
#   C[t,d] = sum_e A[e,t] * enc[e,d]
B, S, D = 8, 2048, 1024
P = 128
EB = S // P   # 16 e-blocks
TB = S // P   # 16 t-blocks
DC = D // P   # 8 d-chunks (contraction for scores)
TC = S // 512 # 4 t-chunks of 512 (matmul free-dim limit)

_NC_CACHE = None


def _build():
    import concourse.bacc as bacc
    import concourse.tile as tile
    from concourse import mybir
    from concourse.masks import make_identity

    F32 = mybir.dt.float32
    F16 = mybir.dt.float16

    nc = bacc.Bacc("TRN2", target_bir_lowering=False, debug=False, num_devices=B)
    enc = nc.declare_dram_parameter("enc_outputs", [S, D], F32, isOutput=False)
    dec = nc.declare_dram_parameter("dec_outputs", [S, D], F32, isOutput=False)
    out = nc.declare_dram_parameter("out", [S, D], F32, isOutput=True)

    with tile.TileContext(nc) as tc:
        with (
            tc.tile_pool(name="const", bufs=1) as const_pool,
            tc.tile_pool(name="encT", bufs=1) as encT_pool,
            tc.tile_pool(name="decT", bufs=1) as decT_pool,
            tc.tile_pool(name="encn", bufs=1) as encn_pool,
            tc.tile_pool(name="decn", bufs=4) as decn_pool,
            tc.tile_pool(name="pmat", bufs=1) as p_pool,
            tc.tile_pool(name="stats", bufs=4) as stats_pool,
            tc.tile_pool(name="ostage", bufs=3) as out_pool,
        ):
            ident = const_pool.tile([P, P], F16, name="ident")
            make_identity(nc, ident)

            decT = [decT_pool.tile([P, S], F16, name=f"decT{d}") for d in range(DC)]
            encT = [encT_pool.tile([P, S], F16, name=f"encT{d}") for d in range(DC)]
            encn = [encn_pool.tile([P, D], F16, name=f"encn{e}") for e in range(EB)]
            pmat = [p_pool.tile([P, S], F16, name=f"p{e}") for e in range(EB)]

            # Cast loads: f32 DRAM -> fp16 SBUF via SWDGE cast-DMA, one
            # 128-row block at a time. dec blocks first (phase B needs all of
            # decT before its first matmul), enc chases.
            dec_tiles = []
            for t in range(TB):
                dtile = decn_pool.tile([P, D], F16, name="decn", tag="decn")
                nc.gpsimd.dma_start(out=dtile[:], in_=dec[t * P : (t + 1) * P, :])
                dec_tiles.append(dtile)
            for e in range(EB):
                nc.gpsimd.dma_start(out=encn[e][:], in_=enc[e * P : (e + 1) * P, :])

            with tc.tile_pool(name="psum_s", bufs=2, space="PSUM") as psum_s:
                # dec transposes on the PE: decT[d][:, t*P:(t+1)*P] = decn[t][:, d*P:(d+1)*P].T
                # Transpose outputs borrow the s_ps tag's PSUM slots (fp16 view,
                # 8 x 256B slices land in the slot's first bank).
                for t in range(TB):
                    tp = psum_s.tile([P, 2 * S], F16, tag="s_ps", name=f"tpd{t}")
                    for d in range(DC):
                        c0 = (d % 4) * 1024 + (d // 4) * P
                        nc.tensor.transpose(
                            tp[:, c0 : c0 + P],
                            dec_tiles[t][:, d * P : (d + 1) * P],
                            ident,
                        )
                        nc.any.tensor_copy(
                            out=decT[d][:, t * P : (t + 1) * P],
                            in_=tp[:, c0 : c0 + P],
                        )

                # Phase B: scores + softmax per e-block, with on-demand enc
                # transposes riding the same PE queue just ahead of the matmuls.
                for e in range(EB):
                    tpe = psum_s.tile([P, 2 * S], F16, tag="s_ps", name=f"tpe{e}")
                    for d in range(DC):
                        c0 = (d % 4) * 1024 + (d // 4) * P
                        nc.tensor.transpose(
                            tpe[:, c0 : c0 + P],
                            encn[e][:, d * P : (d + 1) * P],
                            ident,
                        )
                        nc.any.tensor_copy(
                            out=encT[d][:, e * P : (e + 1) * P],
                            in_=tpe[:, c0 : c0 + P],
                        )
                    s_ps = psum_s.tile([P, S], F32, tag="s_ps", name=f"s_ps{e}")
                    for d in range(DC):
                        for t in range(TC):
                            nc.tensor.matmul(
                                s_ps[:, t * 512 : (t + 1) * 512],
                                lhsT=encT[d][:, e * P : (e + 1) * P],
                                rhs=decT[d][:, t * 512 : (t + 1) * 512],
                                start=(d == 0),
                                stop=(d == DC - 1),
                            )
                    negmax = stats_pool.tile([P, 1], F32, name="negmax")
                    nc.vector.reduce_max(
                        out=negmax, in_=s_ps[:], axis=mybir.AxisListType.X, negate=True
                    )
                    z = stats_pool.tile([P, 1], F32, name="z")
                    nc.scalar.activation(
                        out=pmat[e][:],
                        in_=s_ps[:],
                        func=mybir.ActivationFunctionType.Exp,
                        bias=negmax,
                        scale=1.0,
                        accum_out=z,
                    )
                    zinv = stats_pool.tile([P, 1], F32, name="zinv")
                    nc.vector.reciprocal(zinv, z)
                    # encn[e] <- enc[e] / Z[e]  (per-partition scalar, fp16 out)
                    nc.vector.tensor_scalar_mul(encn[e][:], encn[e][:], zinv)

            # Phase C: context C[t,:] = sum_e P[e,t] * encZ[e,:]
            with tc.tile_pool(name="psum_c", bufs=2, space="PSUM") as psum_c:
                for t in range(TB):
                    c_ps = psum_c.tile([P, D], F32, name="c_ps")
                    for e in range(EB):
                        for hf in range(2):
                            nc.tensor.matmul(
                                c_ps[:, hf * 512 : (hf + 1) * 512],
                                lhsT=pmat[e][:, t * P : (t + 1) * P],
                                rhs=encn[e][:, hf * 512 : (hf + 1) * 512],
                                start=(e == 0),
                                stop=(e == EB - 1),
                            )
                    o_t = out_pool.tile([P, D], F32, name="o_t")
                    nc.any.tensor_copy(out=o_t[:], in_=c_ps[:])
                    nc.scalar.dma_start(out=out[t * P : (t + 1) * P, :], in_=o_t[:])

    nc.compile()
    return nc


def _get_nc():
    global _NC_CACHE
    if _NC_CACHE is None:
        _NC_CACHE = _build()
    return _NC_CACHE


def kernel(enc_outputs, dec_outputs, _want_results=False, **_ignored):
    from concourse.bass_utils import run_bass_kernel_spmd

    nc = _get_nc()
    enc_outputs = np.asarray(enc_outputs, dtype=np.float32)
    dec_outputs = np.asarray(dec_outputs, dtype=np.float32)
    in_maps = [
        {
            "enc_outputs": np.ascontiguousarray(enc_outputs[b]),
            "dec_outputs": np.ascontiguousarray(dec_outputs[b]),
        }
        for b in range(B)
    ]
    res = run_bass_kernel_spmd(nc, in_maps, core_ids=list(range(B)))
    out = np.stack([res.results[b]["out"] for b in range(B)], axis=0)
    if _want_results:
        return out, res
    return out


# revision 9
# speedup vs baseline: 1.3716x; 1.3716x over previous
import sys

for _p in ("/opt/trn_rl_repo", "/root/.axon_site/_ro/trn_rl_repo"):
    if _p not in sys.path:
        sys.path.append(_p)

import numpy as np

# Problem: B=8 batches of cross-attention-like softmax matmul, one batch per core.
#   S[e,t] = sum_d enc[e,d] * dec[t,d]
#   A = softmax(S, axis=t)
#   C[t,d] = sum_e A[e,t] * enc[e,d]
B, S, D = 8, 2048, 1024
P = 128
EB = S // P   # 16 e-blocks
TB = S // P   # 16 t-blocks
DC = D // P   # 8 d-chunks (contraction for scores)
TC = S // 512 # 4 t-chunks of 512 (matmul free-dim limit)

_NC_CACHE = None


def _build():
    import concourse.bacc as bacc
    import concourse.tile as tile
    from concourse import mybir
    from concourse.masks import make_identity

    F32 = mybir.dt.float32
    F16 = mybir.dt.float16

    nc = bacc.Bacc("TRN2", target_bir_lowering=False, debug=False, num_devices=B)
    enc = nc.declare_dram_parameter("enc_outputs", [S, D], F32, isOutput=False)
    dec = nc.declare_dram_parameter("dec_outputs", [S, D], F32, isOutput=False)
    out = nc.declare_dram_parameter("out", [S, D], F32, isOutput=True)

    with tile.TileContext(nc) as tc:
        with (
            tc.tile_pool(name="const", bufs=1) as const_pool,
            tc.tile_pool(name="bigT", bufs=1) as bigT_pool,
            tc.tile_pool(name="encn", bufs=1) as encn_pool,
            tc.tile_pool(name="decn", bufs=4) as decn_pool,
            tc.tile_pool(name="pmat", bufs=1) as p_pool,
            tc.tile_pool(name="stats", bufs=4) as stats_pool,
            tc.tile_pool(name="ostage", bufs=3) as out_pool,
        ):
            ident = const_pool.tile([P, P], F16, name="ident")
            make_identity(nc, ident)

            # d-major transposed operands, one big tile each:
            # encT[:, d*S + e*P + j] = enc[e*P + j, d*P + dd]  (dd = partition)
            encTbig = bigT_pool.tile([P, DC * S], F16, name="encTbig")
            decTbig = bigT_pool.tile([P, DC * S], F16, name="decTbig")
            encn = [encn_pool.tile([P, D], F16, name=f"encn{e}") for e in range(EB)]
            pmat = [p_pool.tile([P, S], F16, name=f"p{e}") for e in range(EB)]

            # Cast loads (f32 DRAM -> fp16 SBUF, SWDGE cast-DMA), dec/enc
            # blocks interleaved so the PE transpose stream can chase them.
            dec_tiles = []
            for t in range(TB):
                dtile = decn_pool.tile([P, D], F16, name="decn", tag="decn")
                nc.gpsimd.dma_start(out=dtile[:], in_=dec[t * P : (t + 1) * P, :])
                dec_tiles.append(dtile)
                nc.gpsimd.dma_start(
                    out=encn[t][:], in_=enc[t * P : (t + 1) * P, :]
                )

            with tc.tile_pool(name="psum_s", bufs=2, space="PSUM") as psum_s:
                # All 256 [128,128] PE transposes up front, in groups of 8
                # with ONE wide eviction per group. Transpose outputs borrow
                # the s_ps tag's PSUM slots (fp16, one bank per group).
                for blk in range(TB):
                    for which, src, tgt in (
                        ("d", dec_tiles[blk], decTbig),
                        ("e", encn[blk], encTbig),
                    ):
                        tp = psum_s.tile([P, D], F16, tag="s_ps", name=f"tp{which}{blk}")
                        for d in range(DC):
                            nc.tensor.transpose(
                                tp[:, d * P : (d + 1) * P],
                                src[:, d * P : (d + 1) * P],
                                ident,
                            )
                        src3 = tp[:].rearrange("p (d s) -> p d s", d=DC)
                        dst3 = tgt[:].rearrange("p (d s) -> p d s", d=DC)[
                            :, :, blk * P : (blk + 1) * P
                        ]
                        nc.any.tensor_copy(out=dst3, in_=src3)

                # Phase B: pure-matmul scores + softmax per e-block.
                for e in range(EB):
                    s_ps = psum_s.tile([P, S], F32, tag="s_ps", name=f"s_ps{e}")
                    for d in range(DC):
                        for t in range(TC):
                            nc.tensor.matmul(
                                s_ps[:, t * 512 : (t + 1) * 512],
                                lhsT=encTbig[:, d * S + e * P : d * S + (e + 1) * P],
                                rhs=decTbig[:, d * S + t * 512 : d * S + (t + 1) * 512],
                                start=(d == 0),
                                stop=(d == DC - 1),
                            )
                    negmax = stats_pool.tile([P, 1], F32, name="negmax")
                    nc.vector.reduce_max(
                        out=negmax, in_=s_ps[:], axis=mybir.AxisListType.X, negate=True
                    )
                    z = stats_pool.tile([P, 1], F32, name="z")
                    nc.scalar.activation(
                        out=pmat[e][:],
                        in_=s_ps[:],
                        func=mybir.ActivationFunctionType.Exp,
                        bias=negmax,
                        scale=1.0,
                        accum_out=z,
                    )
                    zinv = stats_pool.tile([P, 1], F32, name="zinv")
                    nc.vector.reciprocal(zinv, z)
                    # encn[e] <- enc[e] / Z[e]  (per-partition scalar, fp16 out)
                    nc.vector.tensor_scalar_mul(encn[e][:], encn[e][:], zinv)

            # Phase C: context C[t,:] = sum_e P[e,t] * encZ[e,:]
            with tc.tile_pool(name="psum_c", bufs=2, space="PSUM") as psum_c:
                for t in range(TB):
                    c_ps = psum_c.tile([P, D], F32, name="c_ps")
                    for e in range(EB):
                        for hf in range(2):
                            nc.tensor.matmul(
                                c_ps[:, hf * 512 : (hf + 1) * 512],
                                lhsT=pmat[e][:, t * P : (t + 1) * P],
                                rhs=encn[e][:, hf * 512 : (hf + 1) * 512],
                                start=(e == 0),
                                stop=(e == EB - 1),
                            )
                    o_t = out_pool.tile([P, D], F32, name="o_t")
                    for hf in range(2):
                        nc.any.tensor_copy(
                            out=o_t[:, hf * 512 : (hf + 1) * 512],
                            in_=c_ps[:, hf * 512 : (hf + 1) * 512],
                        )
                        nc.scalar.dma_start(
                            out=out[t * P : (t + 1) * P, hf * 512 : (hf + 1) * 512],
                            in_=o_t[:, hf * 512 : (hf + 1) * 512],
                        )

    nc.compile()
    return nc


def _get_nc():
    global _NC_CACHE
    if _NC_CACHE is None:
        _NC_CACHE = _build()
    return _NC_CACHE


def kernel(enc_outputs, dec_outputs, _want_results=False, **_ignored):
    from concourse.bass_utils import run_bass_kernel_spmd

    nc = _get_nc()
    enc_outputs = np.asarray(enc_outputs, dtype=np.float32)
    dec_outputs = np.asarray(dec_outputs, dtype=np.float32)
    in_maps = [
        {
            "enc_outputs": np.ascontiguousarray(enc_outputs[b]),
            "dec_outputs": np.ascontiguousarray(dec_outputs[b]),
        }
        for b in range(B)
    ]
    res = run_bass_kernel_spmd(nc, in_maps, core_ids=list(range(B)))
    out = np.stack([res.results[b]["out"] for b in range(B)], axis=0)
    if _want_results:
        return out, res
    return out
